# revision 9
# baseline (speedup 1.0000x reference)
"""LocallyConnected2d forward on 8 Trainium2 NeuronCores.

Problem: x[64,224,224], per-location weights[112,112,7,7] (stride 2, pad 3),
bias[112,112] -> out[64,112,112].

out[b,i,j] = sum_{kh,kw} x_pad[b, 2i+kh, 2j+kw] * w[i,j,kh,kw] + bias[i,j]

No weight reuse exists (locally-connected), so the TensorEngine is useless;
the kernel is a DVE (vector engine) elementwise-MAC problem.

Sharding: spatial over output rows i. Core c owns i in [14c, 14c+14), all 64
batch images. This needs no weight replication (vs. batch sharding) and keeps
the batch dim (64) as the free dim of every DVE op.

Layout: SBUF holds x transposed (image columns on partitions). Compute-engine
operands must start at partition 0/32/64/96, so the stride-2 column access
cannot be a partition offset; instead the host pre-builds SEVEN column
stores, one per kw tap:
    xs[j, kwi, r, b] = x_pad[b, row0 + r, 2*j + kwi]
For output partition j and tap (kh,kw): in0 = xs[:, kw, 2*i_l+kh, :].
Weights for fixed (i_l,kh,kw) vary only over j -> per-partition scalar ->
one fused DVE scalar_tensor_tensor per (i_l,kh,kw): acc = x*w + acc.
Bias rides in as the first tap's in1 via a broadcast AP.

All host-side reshapes/transposes are numpy and don't count toward HW time.
"""

import sys

sys.path.insert(0, "/opt/trn_rl_repo")

import numpy as np

import concourse.bass as bass
import concourse.bacc as bacc
import concourse.mybir as mybir
from concourse.tile import TileContext
from concourse.bass_utils import run_bass_kernel_spmd

# Problem constants (hardcoded per contract)
B = 64
H = W = 224
KH = KW = 7
SH = SW = 2
PH = PW = 3
NKH = NKW = 112
NCORES = 8
RPC = NKH // NCORES             # 14 output rows per core
SLAB_ROWS = 2 * (RPC - 1) + KH  # 33 x_pad rows per core
RB = SLAB_ROWS * B              # free elems per (kw, j): 2112

F32 = mybir.dt.float32


def _build_nc(n_iters=1):
    nc = bacc.Bacc("TRN2", target_bir_lowering=False, debug=False,
                   num_devices=NCORES)

    # x stores: [kw][j, r*B+b]; one dram tensor per kw so DMA/compute overlap
    x_d = [nc.dram_tensor(f"x{k}", [NKW, RB], F32, kind="ExternalInput")
           for k in range(KW)]
    w_d = nc.dram_tensor("w", [NKW, RPC * KH * KW], F32, kind="ExternalInput")
    b_d = nc.dram_tensor("bias", [NKW, RPC], F32, kind="ExternalInput")
    o_d = nc.dram_tensor("o", [NKW, RPC * B], F32, kind="ExternalOutput")

    with TileContext(nc) as tc:
        with tc.tile_pool(name="p", bufs=1) as pool:
            xs = [pool.tile([NKW, RB], F32, tag=f"x{k}", name=f"xs{k}")
                  for k in range(KW)]
            wt = pool.tile([NKW, RPC * KH * KW], F32, tag="w")
            bt = pool.tile([NKW, RPC], F32, tag="b")
            acc = pool.tile([NKW, RPC * B], F32, tag="acc")

            sink = pool.tile([NKW, 16], F32, tag="sink")

            for _ in range(n_iters):
                nc.gpsimd.dma_start(out=wt[:, :], in_=w_d.ap())
                nc.gpsimd.dma_start(out=bt[:, :], in_=b_d.ap())
                for k in range(KW):
                    nc.gpsimd.dma_start(out=xs[k][:, :], in_=x_d[k].ap())

                # Touch each input tile with a tiny DVE copy so the DMA-wait
                # semaphores land on TensorCopy instructions; the STT ISA
                # struct has too few sync-wait slots to carry them.
                nc.vector.tensor_copy(out=sink[:, 0:1], in_=wt[:, 0:1])
                nc.vector.tensor_copy(out=sink[:, 1:2], in_=bt[:, 0:1])
                for k in range(KW):
                    nc.vector.tensor_copy(out=sink[:, 2 + k:3 + k],
                                          in_=xs[k][:, 0:1])

                for kw in range(KW):
                    for kh in range(KH):
                        first = (kh == 0 and kw == 0)
                        for i_l in range(RPC):
                            row = 2 * i_l + kh
                            in0 = xs[kw][:, row * B:(row + 1) * B]
                            t = i_l * KH * KW + kh * KW + kw
                            sc = wt[:, t:t + 1]
                            out_sl = acc[:, i_l * B:(i_l + 1) * B]
                            if first:
                                in1 = bt[:, i_l:i_l + 1].broadcast_to([NKW, B])
                            else:
                                in1 = out_sl
                            nc.vector.scalar_tensor_tensor(
                                out=out_sl, in0=in0, scalar=sc, in1=in1,
                                op0=mybir.AluOpType.mult,
                                op1=mybir.AluOpType.add)

                nc.gpsimd.dma_start(out=o_d.ap(), in_=acc[:, :])

    nc.compile()
    return nc


def _shard_inputs(x, weights, bias):
    """Host-side prep: pad, transpose, 7 shifted column stores, per-core."""
    x = np.asarray(x, dtype=np.float32)
    weights = np.asarray(weights, dtype=np.float32)
    bias = np.asarray(bias, dtype=np.float32)

    x_pad = np.zeros((B, H + 2 * PH, W + 2 * PW), dtype=np.float32)
    x_pad[:, PH:PH + H, PW:PW + W] = x

    in_maps = []
    for c in range(NCORES):
        r0 = 2 * RPC * c
        slab = x_pad[:, r0:r0 + SLAB_ROWS, :]          # [B, 33, 230]
        t = slab.transpose(2, 1, 0)                    # [230, 33, B]
        m = {}
        for k in range(KW):
            # columns 2j+k for j in [0,112): rows k, k+2, ..., k+222 of t
            m[f"x{k}"] = np.ascontiguousarray(t[k:k + 2 * NKW:2]).reshape(
                NKW, RB)
        m["w"] = np.ascontiguousarray(
            weights[RPC * c:RPC * (c + 1)].transpose(1, 0, 2, 3)
        ).reshape(NKW, RPC * KH * KW)
        m["bias"] = np.ascontiguousarray(bias[RPC * c:RPC * (c + 1)].T)
        in_maps.append(m)
    return in_maps


def _unshard_output(results):
    # results[c]["o"]: [112, 14*64] = (j, i_l, b)
    o_all = np.stack([r["o"].reshape(NKW, RPC, B) for r in results])
    # [c, j, i_l, b] -> [b, c, i_l, j] -> [B, 112, 112]
    return np.ascontiguousarray(o_all.transpose(3, 0, 2, 1)).reshape(B, NKH, NKW)


_NC_CACHE = None


def _get_nc():
    global _NC_CACHE
    if _NC_CACHE is None:
        _NC_CACHE = _build_nc()
    return _NC_CACHE


def kernel(x, weights, bias):
    nc = _get_nc()
    in_maps = _shard_inputs(x, weights, bias)
    res = run_bass_kernel_spmd(nc, in_maps, core_ids=list(range(NCORES)))
    return _unshard_output(res.results)


def benchmark(x, weights, bias, n_big=17, reps=3):
    """Estimate per-iteration device time by differencing wall-clock of a
    1-iteration NEFF vs an n_big-iteration NEFF (amortizes axon RPC, jit and
    transfer overhead, which are identical between the two)."""
    import time

    in_maps = _shard_inputs(x, weights, bias)
    times = {}
    for n in (1, n_big):
        nc = _build_nc(n)
        # warm: compiles NEFF + jit wrap
        run_bass_kernel_spmd(nc, in_maps, core_ids=list(range(NCORES)))
        best = float("inf")
        for _ in range(reps):
            t0 = time.perf_counter()
            res = run_bass_kernel_spmd(nc, in_maps,
                                       core_ids=list(range(NCORES)))
            best = min(best, time.perf_counter() - t0)
        times[n] = best
    per_iter_ns = (times[n_big] - times[1]) / (n_big - 1) * 1e9
    return per_iter_ns, times, _unshard_output(res.results)
